# revision 23
# baseline (speedup 1.0000x reference)
"""Distributed Trainium2 kernel for nn_Attention_68719477187.

RoPE + causal GQA attention (B=2, S=2048, DIM=2048, 32 q heads / 8 kv heads,
head_dim 64) on 8 NeuronCores: DP=2 over batch x TP=4 over head groups.

Per core (b = core//4, G = core%4): 8 q heads / 2 kv heads of batch b.
  1. qkv.T = w{q,k,v}T.T @ x_b.T (contraction over model dim on partitions)
  2. RoPE applied in transposed layout; head_dim pre-permuted (evens, odds)
     on the host so rotation pairs become contiguous 32-partition blocks.
  3. scores.T tiles (k on partitions, q on free) -> exp (no max subtraction;
     scores are O(5) so fp32 exp is safe) -> causal mask by 0/1 multiply ->
     AV matmul with a ones-column appended to V so the softmax denominator
     falls out of the same matmul.
  4. AllGather attention outputs (bf16, chunked per 512 seq positions and
     pipelined behind later attention chunks) within each batch group of 4
     cores, then each core computes a 512-column slice of wo.

Phases are interleaved per sequence chunk sc: qkv(sc) -> attention(qc=sc)
-> AllGather(sc) -> wo(sc-1), so PE matmul work overlaps the ACT-bound
softmax and the collectives.

Compute in bf16 (fp32 PSUM accumulation), output fp32.
"""

import sys

if "/opt/trn_rl_repo" not in sys.path:
    sys.path.insert(0, "/opt/trn_rl_repo")

import numpy as np
import ml_dtypes

from concourse import bacc, tile, mybir
from concourse.bass_utils import run_bass_kernel_spmd

BF16 = ml_dtypes.bfloat16

S = 2048          # sequence length
D = 2048          # model dim
HD = 64           # head dim
NQL = 8           # local q heads
NKVL = 2          # local kv heads
QC = 512          # q chunk (matmul free dim)
NSC = S // QC     # 4 seq chunks
NKD = D // 128    # 16 contraction tiles
NKT = S // 128    # 16 key tiles
SCALE = HD ** -0.5

_NC = None


def _build(_no_cc=False):
    import os
    _bufs = os.environ.get("KBUFS", "")  # "mm,st,av,pp" override for tuning
    mm_b, st_b, av_b, pp_b = ([int(v) for v in _bufs.split(",")]
                              if _bufs else [1, 4, 2, 4])
    nc = bacc.Bacc("TRN2", target_bir_lowering=False, debug=False, num_devices=8)
    BF = mybir.dt.bfloat16
    F32 = mybir.dt.float32
    EXP = mybir.ActivationFunctionType.Exp

    xT = nc.declare_dram_parameter("xT", [D, S], BF, isOutput=False)
    wqT = nc.declare_dram_parameter("wqT", [D, NQL * HD], BF, isOutput=False)
    wkT = nc.declare_dram_parameter("wkT", [D, NKVL * HD], BF, isOutput=False)
    wvT = nc.declare_dram_parameter("wvT", [D, NKVL * HD], BF, isOutput=False)
    woT = nc.declare_dram_parameter("woT", [D, 512], BF, isOutput=False)
    cosS = nc.declare_dram_parameter("cosS", [128, S], F32, isOutput=False)
    sinS = nc.declare_dram_parameter("sinS", [128, S], F32, isOutput=False)
    mask = nc.declare_dram_parameter("mask", [128, 4, QC], BF, isOutput=False)
    out = nc.declare_dram_parameter("out", [512, S], F32, isOutput=True)

    with tile.TileContext(nc) as tc:
        with (
            tc.tile_pool(name="wpool", bufs=1) as wpool,
            tc.tile_pool(name="pers", bufs=1) as pers,
            tc.tile_pool(name="dram", bufs=1, space="DRAM") as dram,
            tc.tile_pool(name="xpool", bufs=3) as xpool,
            tc.tile_pool(name="rtmp", bufs=2) as rtmp,
            tc.tile_pool(name="ppool", bufs=pp_b) as ppool,
            tc.tile_pool(name="npool", bufs=2) as npool,
            tc.tile_pool(name="apool", bufs=2) as apool,
            tc.tile_pool(name="agp", bufs=2) as agp,
            tc.tile_pool(name="opool", bufs=2) as opool,
            tc.tile_pool(name="mmps", bufs=mm_b, space="PSUM") as mmps,
            tc.tile_pool(name="stps", bufs=st_b, space="PSUM") as stps,
            tc.tile_pool(name="avps", bufs=av_b, space="PSUM") as avps,
            tc.tile_pool(name="wops", bufs=1, space="PSUM") as wops,
        ):
            # ---- persistent weights / constants (one 3D DMA each) ----
            wq_sb = [wpool.tile([128, NKD // 2, 512], BF, name=f"wq_sb{h}",
                                tag=f"wq_sb{h}") for h in range(2)]
            wk_sb = wpool.tile([128, NKD, 128], BF, name="wk_sb", tag="wk_sb")
            wv_sb = wpool.tile([128, NKD, 128], BF, name="wv_sb", tag="wv_sb")
            wo_sb = wpool.tile([128, NKD, 512], BF, name="wo_sb", tag="wo_sb")
            cos_sb = wpool.tile([128, S], F32, name="cos_sb", tag="cos_sb")
            sin_sb = wpool.tile([128, S], F32, name="sin_sb", tag="sin_sb")
            mask_sb = wpool.tile([128, 4, QC], BF, name="mask_sb", tag="mask_sb")
            wqTr = wqT.rearrange("(k p) c -> p k c", p=128)

            # ---- persistent activations ----
            qT = [[pers.tile([128, QC], BF, name=f"qT_{rt}_{sc}", tag=f"qT_{rt}_{sc}")
                   for sc in range(NSC)] for rt in range(4)]
            kdup = [[pers.tile([128, QC], BF, name=f"kd_{j}_{sc}", tag=f"kd_{j}_{sc}")
                     for sc in range(NSC)] for j in range(NKVL)]
            vaug = [pers.tile([128, 2, 65], BF, name=f"va_{kt}", tag=f"va_{kt}")
                    for kt in range(NKT)]
            ag_in = [dram.tile([NQL * HD, QC], BF, name=f"ag_in_{qc}")
                     for qc in range(NSC)]
            ag_out = [dram.tile([4 * NQL * HD, QC], BF, name=f"ag_out_{qc}")
                      for qc in range(NSC)]

            xTr = xT.rearrange("(k p) s -> p k s", p=128)
            # hoist x loads so later SP-queue DMAs (which wait on the
            # collectives) never block them; the last chunk is prefetched
            # during qkv(2), still ahead of any AG-dependent DMA in SP order.
            # wq / x chunk 0 are loaded in halves so the first matmuls start
            # after ~1MB of DMA instead of 4MB.
            xts = {}

            def load_x(sc):
                halves = []
                for h in range(2):
                    xt = xpool.tile([128, NKD // 2, QC], BF, name="xt", tag="xt")
                    nc.sync.dma_start(
                        xt[:], xTr[:, h * 8:(h + 1) * 8, sc * QC:(sc + 1) * QC])
                    halves.append(xt)
                xts[sc] = halves

            nc.sync.dma_start(wq_sb[0][:], wqTr[:, 0:8, :])
            load_x(0)
            nc.sync.dma_start(wq_sb[1][:], wqTr[:, 8:16, :])
            nc.sync.dma_start(wk_sb[:], wkT.rearrange("(k p) c -> p k c", p=128))
            nc.sync.dma_start(wv_sb[:], wvT.rearrange("(k p) c -> p k c", p=128))
            nc.sync.dma_start(cos_sb[:], cosS[:])
            nc.sync.dma_start(sin_sb[:], sinS[:])
            for sc in range(1, NSC - 1):
                load_x(sc)
            nc.sync.dma_start(mask_sb[:], mask[:])
            nc.sync.dma_start(wo_sb[:], woT.rearrange("(k p) c -> p k c", p=128))

            def qkv_phase(sc):
                if sc == 2:
                    load_x(3)
                xt = xts[sc]
                cslice = cos_sb[:, sc * QC:(sc + 1) * QC]
                sslice = sin_sb[:, sc * QC:(sc + 1) * QC]
                for rt in range(5):  # 0..3: q row tiles; 4: k row tile
                    ps = mmps.tile([128, QC], F32, name="mm_ps", tag="mm_ps")
                    for kd in range(NKD):
                        lhsT = (wq_sb[kd // 8][:, kd % 8, rt * 128:(rt + 1) * 128]
                                if rt < 4 else wk_sb[:, kd, :])
                        nc.tensor.matmul(ps[:], lhsT, xt[kd // 8][:, kd % 8, :],
                                         start=(kd == 0), stop=(kd == NKD - 1))
                    # rope in fp32 (bf16 only at the final q/k write):
                    # out = raw*cos + swap32(raw)*sin_signed
                    raw = rtmp.tile([128, QC], F32, name="raw", tag="raw")
                    nc.scalar.copy(raw[:], ps[:])
                    rot = rtmp.tile([128, QC], F32, name="rot", tag="rot")
                    for b32 in range(4):
                        src = (b32 ^ 1) * 32
                        nc.gpsimd.tensor_copy(rot[b32 * 32:(b32 + 1) * 32, :],
                                              raw[src:src + 32, :])
                    t1 = rtmp.tile([128, QC], F32, name="t1", tag="t1")
                    nc.vector.tensor_mul(t1[:], raw[:], cslice)
                    nc.vector.tensor_mul(rot[:], rot[:], sslice)
                    if rt < 4:
                        nc.vector.tensor_add(qT[rt][sc][:], t1[:], rot[:])
                    else:
                        kr = rtmp.tile([128, QC], BF, name="kr", tag="kr")
                        nc.vector.tensor_add(kr[:], t1[:], rot[:])
                        for j in range(NKVL):
                            src = kr[j * 64:(j + 1) * 64, :]
                            nc.vector.tensor_copy(kdup[j][sc][0:64, :], src)
                            nc.vector.tensor_copy(kdup[j][sc][64:128, :], src)
                # V computed directly in natural (seq, dim) orientation:
                # lhsT = x.T seq-slice, rhs = wv.T -> out (seq, 2*64) + ones col
                for tt in range(4):
                    kt = sc * 4 + tt
                    vp = stps.tile([128, 128], F32, name="st", tag="st")
                    for kd in range(NKD):
                        nc.tensor.matmul(vp[:],
                                         xt[kd // 8][:, kd % 8, tt * 128:(tt + 1) * 128],
                                         wv_sb[:, kd, :],
                                         start=(kd == 0), stop=(kd == NKD - 1))
                    for j in range(NKVL):
                        nc.vector.tensor_copy(vaug[kt][:, j, 0:64],
                                              vp[:, j * 64:(j + 1) * 64])
                        nc.gpsimd.memset(vaug[kt][:, j, 64:65], 1.0)

            def attn_phase(qc):
                # attention outputs staged in one tile: (128, rt, seq-chunk)
                atile = apool.tile([128, 4, QC], BF, name="atile", tag="atile")
                nkt = 4 * (qc + 1)  # causal: only key tiles up to chunk end
                for rt in range(4):  # head pair (2rt, 2rt+1); shared kv head
                    j = rt // 2
                    avs = [avps.tile([65, QC], F32, name="av", tag="av")
                           for _ in range(2)]
                    for kt in range(nkt):
                        kb = (kt % 4) * 128
                        # diagonal k-tiles only need q columns >= 128*m
                        # (everything left of that is strictly above the
                        # causal diagonal); qo is the q-column offset
                        m = kt - 4 * qc
                        qo = 128 * m if m > 0 else 0
                        n = QC - qo
                        ps_pair = []
                        for half in range(2):
                            # operands at partition base 64*half -> the two
                            # K=64 matmuls run in different PE row groups
                            lo, hi = half * 64, half * 64 + 64
                            st = stps.tile([128, QC], F32, name="st", tag="st")
                            nc.tensor.matmul(st[:, 0:n],
                                             kdup[j][kt // 4][lo:hi, kb:kb + 128],
                                             qT[rt][qc][lo:hi, qo:QC],
                                             start=True, stop=True)
                            p = ppool.tile([128, QC], BF, name="p", tag="p")
                            nc.scalar.activation(p[:, 0:n], st[:, 0:n], EXP,
                                                 scale=SCALE)
                            if m >= 0:  # diagonal tile -> triangular 0/1 mask
                                nc.vector.tensor_mul(p[:, 0:n], p[:, 0:n],
                                                     mask_sb[:, 0, 0:n])
                            ps_pair.append(p)
                        for half in range(2):
                            nc.tensor.matmul(avs[half][:, qo:QC],
                                             vaug[kt][:, j, :],
                                             ps_pair[half][:, 0:n],
                                             start=(kt == 0), stop=(kt == nkt - 1))
                    for half in range(2):
                        av = avs[half]
                        recip = npool.tile([1, QC], F32, name="recip", tag="recip")
                        nc.vector.reciprocal(recip[:], av[64:65, :])
                        rb = npool.tile([64, QC], F32, name="rb", tag="rb")
                        nc.gpsimd.partition_broadcast(rb[:], recip[:])
                        nc.vector.tensor_mul(
                            atile[half * 64:(half + 1) * 64, rt, :],
                            av[0:64, :], rb[:])
                nc.gpsimd.dma_start(
                    ag_in[qc].rearrange("(r p) s -> p r s", p=128), atile[:])
                if _no_cc:
                    # sim-only mode: local copy instead of the collective, to
                    # measure compute-schedule quality without the cost
                    # model's (pessimistic) collective pricing
                    for r in range(4):
                        nc.gpsimd.dma_start(
                            ag_out[qc][r * 512:(r + 1) * 512, :], ag_in[qc][:])
                else:
                    nc.gpsimd.collective_compute(
                        "AllGather", mybir.AluOpType.bypass,
                        replica_groups=[[0, 1, 2, 3], [4, 5, 6, 7]],
                        ins=[ag_in[qc].opt()], outs=[ag_out[qc].opt()])

            def wo_phase(qn):
                agr = ag_out[qn].rearrange("(k p) s -> p k s", p=128)
                agt = []
                for h in range(2):
                    t = agp.tile([128, NKD // 2, QC], BF, name="agt", tag="agt")
                    nc.sync.dma_start(t[:], agr[:, h * 8:(h + 1) * 8, :])
                    agt.append(t)
                for oc in range(4):
                    ps = wops.tile([128, QC], F32, name="wo_ps", tag="wo_ps")
                    for kd in range(NKD):
                        nc.tensor.matmul(ps[:], wo_sb[:, kd, oc * 128:(oc + 1) * 128],
                                         agt[kd // 8][:, kd % 8, :],
                                         start=(kd == 0), stop=(kd == NKD - 1))
                    ot = opool.tile([128, QC], F32, name="ot", tag="ot")
                    nc.scalar.copy(ot[:], ps[:])
                    nc.sync.dma_start(out[oc * 128:(oc + 1) * 128,
                                            qn * QC:(qn + 1) * QC], ot[:])

            for sc in range(NSC):
                qkv_phase(sc)
                attn_phase(sc)
                if sc >= 2:
                    wo_phase(sc - 2)
            wo_phase(NSC - 2)
            wo_phase(NSC - 1)

    nc.compile()
    return nc


def _get_nc():
    global _NC
    if _NC is None:
        _NC = _build()
    return _NC


def _prepare_in_maps(x, freqs_cis, wqkv, wo):
    x = np.asarray(x)
    freqs_cis = np.asarray(freqs_cis)
    wqkv = np.asarray(wqkv)
    wo = np.asarray(wo)

    perm = np.concatenate([np.arange(0, HD, 2), np.arange(1, HD, 2)])
    cos = np.ascontiguousarray(freqs_cis[:, :, 0].T)  # (32, S)
    sin = np.ascontiguousarray(freqs_cis[:, :, 1].T)
    cosS = np.ascontiguousarray(np.concatenate([cos, cos, cos, cos], axis=0),
                                dtype=np.float32)
    sinS = np.ascontiguousarray(np.concatenate([-sin, sin, -sin, sin], axis=0),
                                dtype=np.float32)
    p_i = np.arange(128)[:, None]
    f_i = np.arange(QC)[None, :]
    mask = np.stack([(f_i >= p_i + 128 * m) for m in range(4)], axis=1).astype(BF16)

    xTs = [np.ascontiguousarray(x[b].T).astype(BF16) for b in range(2)]

    in_maps = []
    for c in range(8):
        b, G = c // 4, c % 4
        qrows = np.concatenate([(8 * G + h) * HD + perm for h in range(NQL)])
        krows = np.concatenate([D + (2 * G + j) * HD + perm for j in range(NKVL)])
        vrows = np.concatenate([D + 512 + (2 * G + j) * HD + np.arange(HD)
                                for j in range(NKVL)])
        in_maps.append({
            "xT": xTs[b],
            "wqT": np.ascontiguousarray(wqkv[qrows, :].T).astype(BF16),
            "wkT": np.ascontiguousarray(wqkv[krows, :].T).astype(BF16),
            "wvT": np.ascontiguousarray(wqkv[vrows, :].T).astype(BF16),
            "woT": np.ascontiguousarray(wo[512 * G:512 * (G + 1), :].T).astype(BF16),
            "cosS": cosS,
            "sinS": sinS,
            "mask": mask,
        })
    return in_maps


def kernel(x, freqs_cis, wqkv, wo, _trace=False):
    in_maps = _prepare_in_maps(x, freqs_cis, wqkv, wo)
    res = run_bass_kernel_spmd(_get_nc(), in_maps, core_ids=list(range(8)),
                               trace=_trace)

    outf = np.empty((2, S, D), np.float32)
    for c in range(8):
        b, G = c // 4, c % 4
        outf[b, :, 512 * G:512 * (G + 1)] = res.results[c]["out"].T
    if _trace:
        kernel.last_exec_time_ns = res.exec_time_ns
        kernel.last_results = res
    return outf


# revision 41
# speedup vs baseline: 1.0138x; 1.0138x over previous
"""Distributed Trainium2 kernel for nn_Attention_68719477187.

RoPE + causal GQA attention (B=2, S=2048, DIM=2048, 32 q heads / 8 kv heads,
head_dim 64) on 8 NeuronCores: DP=2 over batch x TP=4 over head groups.

Per core (b = core//4, G = core%4): 8 q heads / 2 kv heads of batch b.
  1. qkv.T = w{q,k,v}T.T @ x_b.T (contraction over model dim on partitions)
  2. RoPE applied in transposed layout; head_dim pre-permuted (evens, odds)
     on the host so rotation pairs become contiguous 32-partition blocks.
  3. scores.T tiles (k on partitions, q on free) -> exp (no max subtraction;
     scores are O(5) so fp32 exp is safe) -> causal mask by 0/1 multiply ->
     AV matmul with a ones-column appended to V so the softmax denominator
     falls out of the same matmul.
  4. AllGather attention outputs (bf16, chunked per 512 seq positions and
     pipelined behind later attention chunks) within each batch group of 4
     cores, then each core computes a 512-column slice of wo.

Phases are interleaved per sequence chunk sc: qkv(sc) -> attention(qc=sc)
-> AllGather(sc) -> wo(sc-1), so PE matmul work overlaps the ACT-bound
softmax and the collectives.

Compute in bf16 (fp32 PSUM accumulation), output fp32.
"""

import sys

if "/opt/trn_rl_repo" not in sys.path:
    sys.path.insert(0, "/opt/trn_rl_repo")

import numpy as np
import ml_dtypes

from concourse import bacc, tile, mybir
from concourse.bass_utils import run_bass_kernel_spmd

BF16 = ml_dtypes.bfloat16

S = 2048          # sequence length
D = 2048          # model dim
HD = 64           # head dim
NQL = 8           # local q heads
NKVL = 2          # local kv heads
QC = 512          # q chunk (matmul free dim)
NSC = S // QC     # 4 seq chunks
NKD = D // 128    # 16 contraction tiles
NKT = S // 128    # 16 key tiles
SCALE = HD ** -0.5

_NC = None


def _build(_no_cc=False):
    import os
    _bufs = os.environ.get("KBUFS", "")  # "mm,st,av,pp" override for tuning
    mm_b, st_b, av_b, pp_b = ([int(v) for v in _bufs.split(",")]
                              if _bufs else [1, 2, 2, 3])
    nc = bacc.Bacc("TRN2", target_bir_lowering=False, debug=False, num_devices=8)
    BF = mybir.dt.bfloat16
    F32 = mybir.dt.float32
    EXP = mybir.ActivationFunctionType.Exp

    # all inputs host-staged to per-partition-contiguous SBUF layouts so DMA
    # descriptor counts stay low (SEQ dispatch cost ~ descriptors)
    xS = nc.declare_dram_parameter("xS", [NSC, 128, NKD, QC], BF, isOutput=False)
    wqS = nc.declare_dram_parameter("wqS", [128, NKD, 512], BF, isOutput=False)
    wkS = nc.declare_dram_parameter("wkS", [128, NKD, 128], BF, isOutput=False)
    wvS = nc.declare_dram_parameter("wvS", [128, NKD, 128], BF, isOutput=False)
    woS = nc.declare_dram_parameter("woS", [128, NKD, 512], BF, isOutput=False)
    cosS = nc.declare_dram_parameter("cosS", [128, S], F32, isOutput=False)
    sinS = nc.declare_dram_parameter("sinS", [128, S], F32, isOutput=False)
    mask = nc.declare_dram_parameter("mask", [128, 2, QC], BF, isOutput=False)
    out = nc.declare_dram_parameter("out", [512, S], F32, isOutput=True)

    with tile.TileContext(nc) as tc:
        with (
            tc.tile_pool(name="wpool", bufs=1) as wpool,
            tc.tile_pool(name="pers", bufs=1) as pers,
            tc.tile_pool(name="dram", bufs=1, space="DRAM") as dram,
            tc.tile_pool(name="xpool", bufs=12) as xpool,
            tc.tile_pool(name="rtmp", bufs=2) as rtmp,
            tc.tile_pool(name="ppool", bufs=pp_b) as ppool,
            tc.tile_pool(name="npool", bufs=2) as npool,
            tc.tile_pool(name="apool", bufs=2) as apool,
            tc.tile_pool(name="agp", bufs=4) as agp,
            tc.tile_pool(name="opool", bufs=2) as opool,
            tc.tile_pool(name="mmps", bufs=mm_b, space="PSUM") as mmps,
            tc.tile_pool(name="stps", bufs=st_b, space="PSUM") as stps,
            tc.tile_pool(name="avps", bufs=av_b, space="PSUM") as avps,
            tc.tile_pool(name="wops", bufs=1, space="PSUM") as wops,
        ):
            # ---- persistent weights / constants (one 3D DMA each) ----
            wq_sb = [wpool.tile([128, NKD // 4, 512], BF, name=f"wq_sb{h}",
                                tag=f"wq_sb{h}") for h in range(4)]
            wk_sb = wpool.tile([128, NKD, 128], BF, name="wk_sb", tag="wk_sb")
            wv_sb = wpool.tile([128, NKD, 128], BF, name="wv_sb", tag="wv_sb")
            wo_sb = wpool.tile([128, NKD, 512], BF, name="wo_sb", tag="wo_sb")
            cos_sb = wpool.tile([128, S], F32, name="cos_sb", tag="cos_sb")
            sin_sb = wpool.tile([128, S], F32, name="sin_sb", tag="sin_sb")
            mask_sb = wpool.tile([128, 2, QC], BF, name="mask_sb", tag="mask_sb")

            # ---- persistent activations ----
            qT = [[pers.tile([128, QC], BF, name=f"qT_{rt}_{sc}", tag=f"qT_{rt}_{sc}")
                   for sc in range(NSC)] for rt in range(4)]
            kdup = [[pers.tile([128, QC], BF, name=f"kd_{j}_{sc}", tag=f"kd_{j}_{sc}")
                     for sc in range(NSC)] for j in range(NKVL)]
            vaug = [pers.tile([128, 2, 65], BF, name=f"va_{kt}", tag=f"va_{kt}")
                    for kt in range(NKT)]
            # AG buffers carry (128, rt, c) blocks per rank so the gather
            # readback has 4KB-contiguous runs per partition
            ag_in = [dram.tile([128, 4, QC], BF, name=f"ag_in_{qc}")
                     for qc in range(NSC)]
            ag_out = [dram.tile([512, 4, QC], BF, name=f"ag_out_{qc}")
                      for qc in range(NSC)]

            # hoist x loads so later SP-queue DMAs (which wait on the
            # collectives) never block them; the last chunk is prefetched
            # during qkv(2), still ahead of any AG-dependent DMA in SP order.
            # wq / x chunk 0 are loaded in halves so the first matmuls start
            # after ~1MB of DMA instead of 4MB.
            xts = {}

            def load_x(sc):
                parts = []
                for h in range(4):
                    xt = xpool.tile([128, NKD // 4, QC], BF, name="xt", tag="xt")
                    nc.sync.dma_start(xt[:], xS[sc, :, h * 4:(h + 1) * 4, :])
                    parts.append(xt)
                xts[sc] = parts

            # interleave wq / x quarters so the kd-accumulation of the very
            # first row tile never waits on a distant load
            nc.sync.dma_start(wq_sb[0][:], wqS[:, 0:4, :])
            xts[0] = []
            for h in range(4):
                xt = xpool.tile([128, NKD // 4, QC], BF, name="xt", tag="xt")
                nc.sync.dma_start(xt[:], xS[0, :, h * 4:(h + 1) * 4, :])
                xts[0].append(xt)
                if h < 3:
                    nc.sync.dma_start(wq_sb[h + 1][:],
                                      wqS[:, (h + 1) * 4:(h + 2) * 4, :])
            nc.sync.dma_start(cos_sb[:, 0:QC], cosS[:, 0:QC])
            nc.sync.dma_start(sin_sb[:, 0:QC], sinS[:, 0:QC])
            nc.sync.dma_start(wk_sb[:], wkS[:])
            nc.sync.dma_start(wv_sb[:], wvS[:])
            nc.sync.dma_start(mask_sb[:], mask[:])
            for sc in range(1, NSC - 1):
                load_x(sc)
                nc.sync.dma_start(cos_sb[:, sc * QC:(sc + 1) * QC],
                                  cosS[:, sc * QC:(sc + 1) * QC])
                nc.sync.dma_start(sin_sb[:, sc * QC:(sc + 1) * QC],
                                  sinS[:, sc * QC:(sc + 1) * QC])
            nc.sync.dma_start(cos_sb[:, 3 * QC:], cosS[:, 3 * QC:])
            nc.sync.dma_start(sin_sb[:, 3 * QC:], sinS[:, 3 * QC:])
            nc.sync.dma_start(wo_sb[:], woS[:])

            def qkv_row(sc, rt):
                xt = xts[sc]
                cslice = cos_sb[:, sc * QC:(sc + 1) * QC]
                sslice = sin_sb[:, sc * QC:(sc + 1) * QC]
                if True:  # 0..3: q row tiles; 4: k row tile
                    ps = mmps.tile([128, QC], F32, name="mm_ps", tag="mm_ps")
                    for kd in range(NKD):
                        lhsT = (wq_sb[kd // 4][:, kd % 4, rt * 128:(rt + 1) * 128]
                                if rt < 4 else wk_sb[:, kd, :])
                        nc.tensor.matmul(ps[:], lhsT, xt[kd // 4][:, kd % 4, :],
                                         start=(kd == 0), stop=(kd == NKD - 1))
                    # rope in fp32 (bf16 only at the final q/k write):
                    # out = raw*cos + swap32(raw)*sin_signed
                    raw = rtmp.tile([128, QC], F32, name="raw", tag="raw")
                    nc.scalar.copy(raw[:], ps[:])
                    rot = rtmp.tile([128, QC], F32, name="rot", tag="rot")
                    for b32 in range(4):
                        src = (b32 ^ 1) * 32
                        nc.gpsimd.tensor_copy(rot[b32 * 32:(b32 + 1) * 32, :],
                                              raw[src:src + 32, :])
                    t1 = rtmp.tile([128, QC], F32, name="t1", tag="t1")
                    nc.vector.tensor_mul(t1[:], raw[:], cslice)
                    nc.vector.tensor_mul(rot[:], rot[:], sslice)
                    if rt < 4:
                        nc.vector.tensor_add(qT[rt][sc][:], t1[:], rot[:])
                    else:
                        kr = rtmp.tile([128, QC], BF, name="kr", tag="kr")
                        nc.vector.tensor_add(kr[:], t1[:], rot[:])
                        for j in range(NKVL):
                            src = kr[j * 64:(j + 1) * 64, :]
                            nc.gpsimd.tensor_copy(kdup[j][sc][0:64, :], src)
                            nc.gpsimd.tensor_copy(kdup[j][sc][64:128, :], src)
            def v_block(sc):
                # V computed directly in natural (seq, dim) orientation:
                # lhsT = x.T seq-slice, rhs = wv.T -> out (seq, 2*64) + ones col
                xt = xts[sc]
                for tt in range(4):
                    kt = sc * 4 + tt
                    vp = stps.tile([128, 128], F32, name="st", tag="st")
                    for kd in range(NKD):
                        nc.tensor.matmul(vp[:],
                                         xt[kd // 4][:, kd % 4, tt * 128:(tt + 1) * 128],
                                         wv_sb[:, kd, :],
                                         start=(kd == 0), stop=(kd == NKD - 1))
                    for j in range(NKVL):
                        nc.vector.tensor_copy(vaug[kt][:, j, 0:64],
                                              vp[:, j * 64:(j + 1) * 64])
                        nc.gpsimd.memset(vaug[kt][:, j, 64:65], 1.0)

            def attn_phase(qc, fillers=()):
                # attention outputs staged in one tile: (128, rt, seq-chunk)
                fillers = list(fillers)
                per_slot = (len(fillers) + 3) // 4 if fillers else 0
                atile = apool.tile([128, 4, QC], BF, name="atile", tag="atile")
                nkt = 4 * (qc + 1)  # causal: only key tiles up to chunk end
                for rt in range(4):  # head pair (2rt, 2rt+1); shared kv head
                    # interleave independent matmul work (next chunk's qkv,
                    # lagged wo) so the PE stream has filler during exp waits
                    for _ in range(per_slot):
                        if fillers:
                            fillers.pop(0)()
                    j = rt // 2
                    avs = [avps.tile([65, QC], F32, name="av", tag="av")
                           for _ in range(2)]
                    for kt in range(nkt):
                        kb = (kt % 4) * 128
                        # diagonal k-tiles only need q columns >= 128*m
                        # (everything left of that is strictly above the
                        # causal diagonal); qo is the q-column offset
                        m = kt - 4 * qc
                        qo = 128 * m if m > 0 else 0
                        n = QC - qo
                        # both halves' scores land in one double-bank PSUM
                        # tile so a single wide exp amortizes the ACT
                        # per-instruction overhead
                        st = stps.tile([128, 2, QC], F32, name="st", tag="st")
                        for half in range(1 if _sim_pair else 2):
                            # operands at partition base 64*half -> the two
                            # K=64 matmuls run in different PE row groups
                            lo, hi = half * 64, half * 64 + 64
                            nc.tensor.matmul(st[:, half, 0:n],
                                             kdup[j][kt // 4][lo:hi, kb:kb + 128],
                                             qT[rt][qc][lo:hi, qo:QC],
                                             start=True, stop=True)
                        p = ppool.tile([128, 2, QC], BF, name="p", tag="p")
                        nc.scalar.activation(p[:, :, 0:n], st[:, :, 0:n], EXP,
                                             scale=SCALE)
                        if m >= 0:  # diagonal tile -> triangular 0/1 mask
                            nc.vector.tensor_mul(p[:, :, 0:n], p[:, :, 0:n],
                                                 mask_sb[:, 0:2, 0:n])
                        for half in range(2):
                            nc.tensor.matmul(avs[half][:, qo:QC],
                                             vaug[kt][:, j, :],
                                             p[:, half, 0:n],
                                             start=(kt == 0), stop=(kt == nkt - 1))
                    for half in range(2):
                        av = avs[half]
                        recip = npool.tile([1, QC], F32, name="recip", tag="recip")
                        nc.vector.reciprocal(recip[:], av[64:65, :])
                        rb = npool.tile([64, QC], F32, name="rb", tag="rb")
                        nc.gpsimd.partition_broadcast(rb[:], recip[:])
                        nc.vector.tensor_mul(
                            atile[half * 64:(half + 1) * 64, rt, :],
                            av[0:64, :], rb[:])
                nc.gpsimd.dma_start(ag_in[qc][:], atile[:])
                if _no_cc:
                    # sim-only mode: local copy instead of the collective, to
                    # measure compute-schedule quality without the cost
                    # model's (pessimistic) collective pricing. NO_CC=2 makes
                    # the gather entirely free (lower bracket).
                    nreps = 1 if str(_no_cc) == "2" else 4
                    for r in range(nreps):
                        nc.gpsimd.dma_start(
                            ag_out[qc][r * 128:(r + 1) * 128, :, :], ag_in[qc][:])
                else:
                    nc.gpsimd.collective_compute(
                        "AllGather", mybir.AluOpType.bypass,
                        replica_groups=[[0, 1, 2, 3], [4, 5, 6, 7]],
                        ins=[ag_in[qc].opt()], outs=[ag_out[qc].opt()])

            agts = {}

            def wo_load(qn):
                # (4 ranks * 128p, rt, c) -> (p, rank, rt, c); kd = rank*4+rt
                agr = ag_out[qn].rearrange("(r p) k c -> p r k c", p=128)
                agt = []
                for h in range(2):
                    t = agp.tile([128, 2, 4, QC], BF, name="agt", tag="agt")
                    nc.sync.dma_start(t[:], agr[:, 2 * h:2 * h + 2, :, :])
                    agt.append(t)
                agts[qn] = agt

            def wo_block(qn, oc):
                agt = agts[qn]
                if True:
                    ps = wops.tile([128, QC], F32, name="wo_ps", tag="wo_ps")
                    for kd in range(NKD):
                        nc.tensor.matmul(ps[:], wo_sb[:, kd, oc * 128:(oc + 1) * 128],
                                         agt[kd // 8][:, (kd % 8) // 4, kd % 4, :],
                                         start=(kd == 0), stop=(kd == NKD - 1))
                    ot = opool.tile([128, QC], F32, name="ot", tag="ot")
                    nc.scalar.copy(ot[:], ps[:])
                    nc.sync.dma_start(out[oc * 128:(oc + 1) * 128,
                                          qn * QC:(qn + 1) * QC], ot[:])

            def qkv_units(sc):
                units = []
                if sc == 2:
                    units.append(lambda: load_x(3))
                units += [lambda rt=rt: qkv_row(sc, rt) for rt in range(5)]
                units.append(lambda: v_block(sc))
                return units

            def wo_units(qn):
                units = [lambda: wo_load(qn)]
                units += [lambda oc=oc: wo_block(qn, oc) for oc in range(4)]
                return units

            for sc in range(NSC):
                for u in qkv_units(sc):
                    u()
                attn_phase(sc)
                if sc >= 2:
                    for u in wo_units(sc - 2):
                        u()
                if sc == NSC - 1:
                    for u in wo_units(NSC - 2):
                        u()
            for u in wo_units(NSC - 1):
                u()

    nc.compile()
    return nc


def _get_nc():
    global _NC
    if _NC is None:
        _NC = _build()
    return _NC


def _prepare_in_maps(x, freqs_cis, wqkv, wo):
    x = np.asarray(x)
    freqs_cis = np.asarray(freqs_cis)
    wqkv = np.asarray(wqkv)
    wo = np.asarray(wo)

    perm = np.concatenate([np.arange(0, HD, 2), np.arange(1, HD, 2)])
    cos = np.ascontiguousarray(freqs_cis[:, :, 0].T)  # (32, S)
    sin = np.ascontiguousarray(freqs_cis[:, :, 1].T)
    cosS = np.ascontiguousarray(np.concatenate([cos, cos, cos, cos], axis=0),
                                dtype=np.float32)
    sinS = np.ascontiguousarray(np.concatenate([-sin, sin, -sin, sin], axis=0),
                                dtype=np.float32)
    p_i = np.arange(128)[:, None]
    f_i = np.arange(QC)[None, :]
    tri = (f_i >= p_i)
    mask = np.stack([tri, tri], axis=1).astype(BF16)

    def stage(wt):
        # (D, C) with D = 16*128 -> (128, 16, C), per-partition contiguous
        return np.ascontiguousarray(
            wt.reshape(NKD, 128, wt.shape[1]).transpose(1, 0, 2)).astype(BF16)

    xSs = []
    for b in range(2):
        xt = x[b].T  # (D, S)
        xs = xt.reshape(NKD, 128, NSC, QC).transpose(2, 1, 0, 3)
        xSs.append(np.ascontiguousarray(xs).astype(BF16))

    in_maps = []
    for c in range(8):
        b, G = c // 4, c % 4
        qrows = np.concatenate([(8 * G + h) * HD + perm for h in range(NQL)])
        krows = np.concatenate([D + (2 * G + j) * HD + perm for j in range(NKVL)])
        vrows = np.concatenate([D + 512 + (2 * G + j) * HD + np.arange(HD)
                                for j in range(NKVL)])
        in_maps.append({
            "xS": xSs[b],
            "wqS": stage(wqkv[qrows, :].T),
            "wkS": stage(wqkv[krows, :].T),
            "wvS": stage(wqkv[vrows, :].T),
            "woS": stage(wo[512 * G:512 * (G + 1), :].T),
            "cosS": cosS,
            "sinS": sinS,
            "mask": mask,
        })
    return in_maps


def kernel(x, freqs_cis, wqkv, wo, _trace=False):
    in_maps = _prepare_in_maps(x, freqs_cis, wqkv, wo)
    res = run_bass_kernel_spmd(_get_nc(), in_maps, core_ids=list(range(8)),
                               trace=_trace)

    outf = np.empty((2, S, D), np.float32)
    for c in range(8):
        b, G = c // 4, c % 4
        outf[b, :, 512 * G:512 * (G + 1)] = res.results[c]["out"].T
    if _trace:
        kernel.last_exec_time_ns = res.exec_time_ns
        kernel.last_results = res
    return outf
